# revision 25
# baseline (speedup 1.0000x reference)
"""Trainium2 Bass kernel for nn_ConformalLayers (8-core data-parallel).

Math (reference):
    X = x.reshape(B, 3072).T                         # [3072, B]
    Y = M @ X                                        # [16384, B]
    Y_extra = s * ||X||_col + sum((T @ X) * X, 0)    # [1, B]
    out = (Y / Y_extra).T.reshape(B, 64, 16, 16)

Sharding: batch B=4096 split as 512 columns per core; model caches
replicated. Each core computes out^T rows [512, 16384] locally; host
concatenates.

All GEMMs run in fp8e4m3 with MatmulPerfMode.DoubleRow (2 k-tiles of 128
per instruction at 0.5 cycles/row = 4x the fp16 rate). fp8 quantization
of both operands alone gives ~3.2e-2 rel err (gate is 2e-2), so the main
GEMM adds two residual-correction streams over the first C=10 of 12
k-pairs:

    32*M@X ~= M1@X1  +  dM8@X8  +  M1@dXq        (all fp8 DoubleRow)
      M1  = fp8(32*M)          X1  = fp8(X)
      dM8 = fp8(8*(32M - M1))  X8  = fp8(X1/8)    -> dM8@X8 ~ (32M-M1)@X1
      dXq = fp8(X - X1)                           -> M1@dXq ~ M1@(X-X1)

    scales multiply back to 1 (the fp8 denormal range absorbs dXq's
    small magnitudes), so all three accumulate into one PSUM bank.
    Measured rel err 1.3e-2 on the real inputs.

The quadratic form q = x^T T x uses only the symmetric part S=(T+T^T)/2:
q = 2 x^T U x + sum(diag(S) x^2) with U=triu(S,1), so the device GEMM2
streams only the nonzero upper-triangular k-pairs (42 of 72 tile-pairs,
at fp8 scale 1024). The diagonal term and s*||x|| are folded into a tiny
host-computed [128,4] tensor yb = 1024*(s*||x|| + sum(d x^2)); then
1/Y_extra = 32*recip(2*qv + yb) scales the PSUM eviction (out in fp16).

All DRAM operands are pre-tiled host-side so every DMA is a [128 part x
contiguous-bytes] block.
"""

import os
from contextlib import ExitStack

import numpy as np
import ml_dtypes

import concourse.bass as bass
import concourse.tile as tile
from concourse import bacc, mybir
from concourse import bass_utils

B = 4096
IN_NUMEL = 3072
OUT_NUMEL = 16384
OUT_DIMS = (64, 16, 16)
N_CORES = 8
BC = B // N_CORES            # 512 batch columns per core
P = 128
NB = BC // P                 # 4 batch blocks of 128
NMC = OUT_NUMEL // 512       # 32 m-chunks
KP = IN_NUMEL // 256         # 12 k-pairs (256 rows each)
NKC = IN_NUMEL // 512        # 6 k'-chunks for GEMM2

CM = int(os.environ.get("KERNEL_CM", "9"))    # corrected pairs, M-side
CX = int(os.environ.get("KERNEL_CX", "9"))    # corrected pairs, X-side
T_SUBB = 2 * (CM + CX)                         # xsB subtiles

SCALE_M = 32.0
SCALE_T = 1024.0

E4 = ml_dtypes.float8_e4m3
FP8 = mybir.dt.float8e4
F16 = mybir.dt.float16
F32 = mybir.dt.float32

# GEMM2 upper-triangular pair packing: chunk c uses pairs 0..2c+1
U_NP = [2 * c + 2 for c in range(NKC)]
U_OFF = [sum(U_NP[:c]) for c in range(NKC)]
U_TOT = sum(U_NP)                              # 42

_PROGRAM = None


def _build_program():
    nc = bacc.Bacc(
        "TRN2",
        target_bir_lowering=False,
        debug=False,
        enable_asserts=False,
        num_devices=N_CORES,
        enable_partition_id=False,
    )
    xsa_d = nc.dram_tensor("xsa", (P, 2 * KP, BC), FP8, kind="ExternalInput")
    xsb_d = nc.dram_tensor("xsb", (P, T_SUBB, BC), FP8, kind="ExternalInput")
    xh_d = nc.dram_tensor("xh", (P, NB, IN_NUMEL), FP8, kind="ExternalInput")
    yb_d = nc.dram_tensor("yb", (P, NB), F32, kind="ExternalInput")
    m1_d = nc.dram_tensor("m1", (NMC, P, KP, 2, 512), FP8, kind="ExternalInput")
    dm_d = nc.dram_tensor("dm", (NMC, P, CM, 2, 512), FP8, kind="ExternalInput")
    u8_d = nc.dram_tensor("u8", (U_TOT, P, 2, 512), FP8, kind="ExternalInput")
    out_d = nc.dram_tensor("out", (BC, OUT_NUMEL), F16, kind="ExternalOutput")

    Alu = mybir.AluOpType
    DR = mybir.MatmulPerfMode.DoubleRow

    m1_ap = m1_d.ap()
    dm_ap = dm_d.ap()
    u8_ap = u8_d.ap().rearrange("n p two i -> p n two i")
    out_ap = out_d.ap().rearrange("(t p) m -> p t m", p=P)

    with tile.TileContext(nc) as tc:
        with ExitStack() as ctx:
            small = ctx.enter_context(tc.tile_pool(name="small", bufs=1))
            xs_pool = ctx.enter_context(tc.tile_pool(name="xsp", bufs=1))
            xh_pool = ctx.enter_context(tc.tile_pool(name="xhp", bufs=1))
            u_pool = ctx.enter_context(tc.tile_pool(name="up", bufs=1))
            mm_pool = ctx.enter_context(tc.tile_pool(name="mmp", bufs=3))
            scr_pool = ctx.enter_context(tc.tile_pool(name="scr", bufs=2))
            out_pool = ctx.enter_context(tc.tile_pool(name="outp", bufs=6))
            psg2 = ctx.enter_context(tc.psum_pool(name="psg2", bufs=3))
            psg1 = ctx.enter_context(tc.psum_pool(name="psg1", bufs=5))

            qp = small.tile([P, NB * NKC], F32)
            qv = small.tile([P, NB], F32)
            ye = small.tile([P, NB], F32)
            rt = small.tile([P, NB], F32)
            rt32 = small.tile([P, NB], F32)
            yb_t = small.tile([P, NB], F32)

            xsa_t = xs_pool.tile([P, 2 * KP, BC], FP8)
            xsb_t = xs_pool.tile([P, T_SUBB, BC], FP8)
            xh_t = xh_pool.tile([P, NB, IN_NUMEL], FP8)
            u_tiles = [
                u_pool.tile([P, U_NP[c], 2, 512], FP8, tag=f"u{c}", name=f"u{c}")
                for c in range(NKC)
            ]

            # Early DMAs, in the order they should hit the wire: each lands
            # just before the in-order PE stream needs it. xsa/m1_0 stream
            # as interleaved thirds so the chunk-0 main matmuls start ~5us
            # in; GEMM2 u-tiles stream largest-first (the chunk loop below
            # runs c=5..0 to match).
            m1_cur = mm_pool.tile([P, KP, 2, 512], FP8, tag="m1")
            dm_cur = mm_pool.tile([P, CM, 2, 512], FP8, tag="dm")
            # xsa + the largest GEMM2 u-tile stream first: GEMM2 chunk 5
            # needs only these two, so the PE starts ~6.5us in, before the
            # m1 stream has landed.
            nc.sync.dma_start(xsa_t[:, 0:12], xsa_d.ap()[:, 0:12])
            nc.sync.dma_start(xsa_t[:, 12:24], xsa_d.ap()[:, 12:24])
            def dma_u(c, lo, hi):
                nc.sync.dma_start(
                    u_tiles[c][:, lo:hi], u8_ap[:, U_OFF[c] + lo:U_OFF[c] + hi]
                )

            dma_u(5, 0, 6)
            dma_u(5, 6, 12)
            for lo, hi in ((0, 4), (4, 8), (8, 12)):
                nc.sync.dma_start(m1_cur[:, lo:hi], m1_ap[0][:, lo:hi])
            nc.sync.dma_start(xh_t[:, 0:2], xh_d.ap()[:, 0:2])
            dma_u(4, 0, 5)
            nc.sync.dma_start(xh_t[:, 2:4], xh_d.ap()[:, 2:4])
            dma_u(4, 5, 10)
            nc.sync.dma_start(xsb_t[:, 2 * CM:], xsb_d.ap()[:, 2 * CM:])  # dXq
            dma_u(3, 0, 8)
            dma_u(2, 0, 6)
            dma_u(1, 0, 4)
            dma_u(0, 0, 2)
            nc.sync.dma_start(xsb_t[:, 0:2 * CM], xsb_d.ap()[:, 0:2 * CM])  # X8
            nc.sync.dma_start(dm_cur[:], dm_ap[0])
            nc.sync.dma_start(yb_t[:], yb_d.ap())
            m1_nxt = mm_pool.tile([P, KP, 2, 512], FP8, tag="m1")
            dm_nxt = mm_pool.tile([P, CM, 2, 512], FP8, tag="dm")
            nc.sync.dma_start(m1_nxt[:, 0:6], m1_ap[1][:, 0:6])
            nc.sync.dma_start(m1_nxt[:, 6:12], m1_ap[1][:, 6:12])
            nc.sync.dma_start(dm_nxt[:], dm_ap[1])

            # GEMM2: qp[b,c] = sum((U^T X)[:,512c:512c+512] * x_nat)
            def gemm2_chunk(c):
                u_t = u_tiles[c]
                for b in range(NB):
                    ps = psg2.tile([P, 512], F32, tag="g2", name="psg2t")
                    npair = U_NP[c]
                    for j in range(npair):
                        nc.tensor.matmul(
                            ps[:],
                            xsa_t[:, 2 * j:2 * j + 2, b * P:(b + 1) * P],
                            u_t[:, j],
                            start=(j == 0),
                            stop=(j == npair - 1),
                            perf_mode=DR,
                        )
                    tmp = scr_pool.tile([P, 512], F32, tag="red", name="red")
                    nc.vector.tensor_tensor_reduce(
                        out=tmp[:], in0=ps[:],
                        in1=xh_t[:, b, c * 512:(c + 1) * 512],
                        scale=2.0, scalar=0.0,
                        op0=Alu.mult, op1=Alu.add,
                        accum_out=qp[:, b * NKC + c:b * NKC + c + 1],
                    )

            gemm2_chunk(5)

            # G1 chunk 0, main-term matmuls (j-major to match the arriving
            # m1 thirds). The 4 PSUM groups stay open (no stop) until the
            # corrections are appended between/after the GEMM2 chunks.
            ps0 = [
                psg1.tile([P, 512], F32, tag="g1", name=f"ps0_{b}")
                for b in range(NB)
            ]
            for j in range(KP):
                for b in range(NB):
                    nc.tensor.matmul(
                        ps0[b][:],
                        xsa_t[:, 2 * j:2 * j + 2, b * P:(b + 1) * P],
                        m1_cur[:, j],
                        start=(j == 0), stop=False, perf_mode=DR,
                    )

            gemm2_chunk(4)
            # chunk-0 X-side corrections: slot between the big and small
            # GEMM2 chunks to match the DMA arrival order (xsb lands after
            # u5/u4, before u3..u0).
            for b in range(NB):
                bsl = slice(b * P, (b + 1) * P)
                for j in range(CX):
                    nc.tensor.matmul(
                        ps0[b][:],
                        xsb_t[:, 2 * CM + 2 * j:2 * CM + 2 * j + 2, bsl],
                        m1_cur[:, j],
                        start=False, stop=False, perf_mode=DR,
                    )
            for c in (3, 2, 1, 0):
                gemm2_chunk(c)

            # 1/Y_extra (at combined scale): rt32 = 32 / (qv + yb); the 2x on
            # the strict-upper quadratic term is folded into the reducers.
            for b in range(NB):
                nc.vector.tensor_reduce(
                    qv[:, b:b + 1], qp[:, b * NKC:(b + 1) * NKC],
                    mybir.AxisListType.X, Alu.add,
                )
            nc.vector.scalar_tensor_tensor(
                out=ye[:], in0=qv[:], scalar=1.0, in1=yb_t[:],
                op0=Alu.mult, op1=Alu.add,
            )
            nc.vector.reciprocal(rt[:], ye[:])
            nc.vector.tensor_scalar_mul(rt32[:], rt[:], float(SCALE_M))

            # GEMM1: 32 m-chunks x 4 b-subtiles; 12 main + CX + CM fp8-DR
            # matmuls accumulate into one PSUM bank, evicted as out*rt32.
            def corr1_and_evict(ps, b, mu, dm_t):
                bsl = slice(b * P, (b + 1) * P)
                for j in range(CM):           # dM8 @ X8 (M-side residual)
                    nc.tensor.matmul(
                        ps[:], xsb_t[:, 2 * j:2 * j + 2, bsl], dm_t[:, j],
                        start=False, stop=(j == CM - 1), perf_mode=DR,
                    )
                ot = out_pool.tile([P, 512], F16, name="ot")
                if b % 2:    # split evictions across DVE and the idle ACT
                    nc.scalar.mul(ot[:], ps[:], rt32[:, b:b + 1])
                else:
                    nc.vector.tensor_scalar_mul(ot[:], ps[:], rt32[:, b:b + 1])
                nc.sync.dma_start(out_ap[:, b, mu * 512:(mu + 1) * 512], ot[:])

            def corr_and_evict(ps, b, mu, m1_t, dm_t):
                bsl = slice(b * P, (b + 1) * P)
                for j in range(CX):           # M1 @ dXq (X-side residual)
                    nc.tensor.matmul(
                        ps[:],
                        xsb_t[:, 2 * CM + 2 * j:2 * CM + 2 * j + 2, bsl],
                        m1_t[:, j],
                        start=False, stop=False, perf_mode=DR,
                    )
                corr1_and_evict(ps, b, mu, dm_t)

            for b in range(NB):               # finish chunk 0
                corr1_and_evict(ps0[b], b, 0, dm_cur)
            m1_cur, dm_cur = m1_nxt, dm_nxt

            for mu in range(1, NMC):
                if mu + 1 < NMC:
                    m1_nxt = mm_pool.tile([P, KP, 2, 512], FP8, tag="m1")
                    dm_nxt = mm_pool.tile([P, CM, 2, 512], FP8, tag="dm")
                    nc.sync.dma_start(m1_nxt[:], m1_ap[mu + 1])
                    nc.sync.dma_start(dm_nxt[:], dm_ap[mu + 1])
                for b in range(NB):
                    bsl = slice(b * P, (b + 1) * P)
                    ps = psg1.tile([P, 512], F32, tag="g1")
                    for j in range(KP):
                        nc.tensor.matmul(
                            ps[:], xsa_t[:, 2 * j:2 * j + 2, bsl], m1_cur[:, j],
                            start=(j == 0), stop=False, perf_mode=DR,
                        )
                    corr_and_evict(ps, b, mu, m1_cur, dm_cur)
                if mu + 1 < NMC:
                    m1_cur, dm_cur = m1_nxt, dm_nxt

    nc.compile()
    return nc


def get_program():
    global _PROGRAM
    if _PROGRAM is None:
        _PROGRAM = _build_program()
    return _PROGRAM


def _f8(a):
    return np.asarray(a, dtype=np.float32).astype(E4)


def make_in_maps(x, cached_matrix, cached_matrix_extra, cached_tensor_extra):
    xf = np.ascontiguousarray(np.asarray(x, dtype=np.float32).reshape(B, IN_NUMEL))
    s = float(np.asarray(cached_matrix_extra).reshape(-1)[0])

    # --- replicated model-cache tensors ---
    MT = np.ascontiguousarray(np.asarray(cached_matrix, dtype=np.float32).T)
    M32 = SCALE_M * MT
    M1 = _f8(M32)                                    # [3072, 16384] fp8
    dM8 = _f8(8.0 * (M32 - M1.astype(np.float32)))[: 256 * CM]
    # pre-tile: k = j*256 + tw*128 + p, m = mu*512 + i -> [mu, p, j, tw, i]
    m1_t = np.ascontiguousarray(
        M1.reshape(KP, 2, P, NMC, 512).transpose(3, 2, 0, 1, 4)
    )
    dm_t = np.ascontiguousarray(
        dM8.reshape(CM, 2, P, NMC, 512).transpose(3, 2, 0, 1, 4)
    )

    T0 = np.asarray(cached_tensor_extra, dtype=np.float32)
    S = 0.5 * (T0 + T0.T)
    d = np.diag(S).astype(np.float64).copy()
    U8 = _f8(SCALE_T * np.triu(S, 1))                # [3072, 3072] fp8
    u_parts = []
    for c in range(NKC):
        for j in range(U_NP[c]):
            blk = U8[256 * j:256 * (j + 1), 512 * c:512 * (c + 1)]
            u_parts.append(blk.reshape(2, P, 512).transpose(1, 0, 2))
    u8_t = np.ascontiguousarray(np.stack(u_parts, axis=0))  # [42, 128, 2, 512]

    # --- per-core batch-sharded tensors ---
    x64 = xf.astype(np.float64)
    yb_full = SCALE_T * (
        s * np.sqrt(np.sum(x64 * x64, axis=1))
        + np.sum(x64 * x64 * d[None, :], axis=1)
    ).astype(np.float64)                              # [B]

    in_maps = []
    for cidx in range(N_CORES):
        sl = slice(cidx * BC, (cidx + 1) * BC)
        Xc = np.ascontiguousarray(xf[sl].T)          # [3072, 512] f32
        X1 = _f8(Xc)
        X8 = _f8(X1.astype(np.float32) / 8.0)[: 256 * CM]
        dXq = _f8(Xc - X1.astype(np.float32))[: 256 * CX]
        xsa = np.ascontiguousarray(
            X1.reshape(2 * KP, P, BC).transpose(1, 0, 2)
        )
        xsb = np.ascontiguousarray(
            np.concatenate([X8, dXq], axis=0).reshape(T_SUBB, P, BC).transpose(1, 0, 2)
        )
        xh = np.ascontiguousarray(
            _f8(xf[sl]).reshape(NB, P, IN_NUMEL).transpose(1, 0, 2)
        )
        yb = np.ascontiguousarray(
            yb_full[sl].astype(np.float32).reshape(NB, P).T
        )
        in_maps.append({
            "xsa": xsa,
            "xsb": xsb,
            "xh": xh,
            "yb": yb,
            "m1": m1_t,
            "dm": dm_t,
            "u8": u8_t,
        })
    return in_maps


_AXON_EXEC = None
_SHARDED_INPUTS = {"xsa", "xsb", "xh", "yb"}


def _build_axon_exec():
    """Staged PJRT runner for the axon path.

    run_bass_kernel_spmd's axon redirect concatenates all per-core inputs into
    single giant host arrays for the replicated model caches, which hits a
    pathologically slow transfer path in the relay. Instead we stage shards/
    replicas with individually-sized device_puts and run the same bass_exec
    custom call through shard_map ourselves.
    """
    import jax
    from jax.sharding import Mesh, NamedSharding, PartitionSpec
    from jax.experimental.shard_map import shard_map
    from concourse import bass2jax

    nc = get_program()
    bass2jax.install_neuronx_cc_hook()

    in_names, out_names, out_avals = [], [], []
    for alloc in nc.m.functions[0].allocations:
        if not isinstance(alloc, mybir.MemoryLocationSet):
            continue
        name = alloc.memorylocations[0].name
        if alloc.kind == "ExternalInput":
            in_names.append(name)
        elif alloc.kind == "ExternalOutput":
            out_names.append(name)
            out_avals.append(
                jax.core.ShapedArray(
                    tuple(alloc.tensor_shape), mybir.dt.np(alloc.dtype)
                )
            )
    all_in_names = in_names + out_names

    def _body(*args):
        outs = bass2jax._bass_exec_p.bind(
            *args,
            out_avals=tuple(out_avals),
            in_names=tuple(all_in_names),
            out_names=tuple(out_names),
            lowering_input_output_aliases=(),
            sim_require_finite=True,
            sim_require_nnan=True,
            nc=nc,
        )
        return tuple(outs)

    devices = jax.devices()[:N_CORES]
    mesh = Mesh(np.asarray(devices), ("core",))
    core_spec = PartitionSpec("core")
    repl_spec = PartitionSpec()
    in_specs = tuple(
        core_spec if n in _SHARDED_INPUTS else repl_spec for n in in_names
    ) + (core_spec,) * len(out_names)
    sharded = jax.jit(
        shard_map(
            _body,
            mesh=mesh,
            in_specs=in_specs,
            out_specs=(core_spec,) * len(out_names),
            check_rep=False,
        ),
        keep_unused=True,
    )

    def stage(in_maps):
        import concurrent.futures as cf

        core_sh = NamedSharding(mesh, core_spec)
        repl_sh = NamedSharding(mesh, repl_spec)

        def stage_one(name):
            if name in _SHARDED_INPUTS:
                glob = np.concatenate([m[name] for m in in_maps], axis=0)
                return jax.device_put(glob, core_sh)
            return jax.device_put(in_maps[0][name], repl_sh)

        with cf.ThreadPoolExecutor(len(in_names)) as ex:
            staged = list(ex.map(stage_one, in_names))
        for st in staged:
            st.block_until_ready()
        zeros = [
            jax.jit(
                lambda a=a: jax.numpy.zeros(
                    (N_CORES * a.shape[0], *a.shape[1:]), a.dtype
                ),
                out_shardings=core_sh,
            )()
            for a in out_avals
        ]
        return staged + zeros

    def execute(staged):
        outs = sharded(*staged)
        jax.block_until_ready(outs)
        return outs

    def run(in_maps):
        return execute(stage(in_maps))

    return {"sharded": sharded, "stage": stage, "execute": execute, "run": run}


def get_axon_exec():
    global _AXON_EXEC
    if _AXON_EXEC is None:
        _AXON_EXEC = _build_axon_exec()
    return _AXON_EXEC


def kernel(x, cached_matrix, cached_matrix_extra, cached_tensor_extra):
    from concourse._compat import axon_active

    in_maps = make_in_maps(x, cached_matrix, cached_matrix_extra, cached_tensor_extra)
    if axon_active():
        outs = get_axon_exec()["run"](in_maps)
        out = np.asarray(outs[0])  # [B, OUT_NUMEL] f16
    else:
        nc = get_program()
        res = bass_utils.run_bass_kernel_spmd(nc, in_maps, core_ids=list(range(N_CORES)))
        out = np.concatenate([r["out"] for r in res.results], axis=0)
    return np.ascontiguousarray(out.astype(np.float32)).reshape(B, *OUT_DIMS)


# revision 28
# speedup vs baseline: 1.0026x; 1.0026x over previous
"""Trainium2 Bass kernel for nn_ConformalLayers (8-core data-parallel).

Math (reference):
    X = x.reshape(B, 3072).T                         # [3072, B]
    Y = M @ X                                        # [16384, B]
    Y_extra = s * ||X||_col + sum((T @ X) * X, 0)    # [1, B]
    out = (Y / Y_extra).T.reshape(B, 64, 16, 16)

Sharding: batch B=4096 split as 512 columns per core; model caches
replicated. Each core computes out^T rows [512, 16384] locally; host
concatenates.

All GEMMs run in fp8e4m3 with MatmulPerfMode.DoubleRow (2 k-tiles of 128
per instruction at 0.5 cycles/row = 4x the fp16 rate). fp8 quantization
of both operands alone gives ~3.2e-2 rel err (gate is 2e-2), so the main
GEMM adds two residual-correction streams over the first C=10 of 12
k-pairs:

    32*M@X ~= M1@X1  +  dM8@X8  +  M1@dXq        (all fp8 DoubleRow)
      M1  = fp8(32*M)          X1  = fp8(X)
      dM8 = fp8(8*(32M - M1))  X8  = fp8(X1/8)    -> dM8@X8 ~ (32M-M1)@X1
      dXq = fp8(X - X1)                           -> M1@dXq ~ M1@(X-X1)

    scales multiply back to 1 (the fp8 denormal range absorbs dXq's
    small magnitudes), so all three accumulate into one PSUM bank.
    Measured rel err 1.3e-2 on the real inputs.

The quadratic form q = x^T T x uses only the symmetric part S=(T+T^T)/2:
q = 2 x^T U x + sum(diag(S) x^2) with U=triu(S,1), so the device GEMM2
streams only the nonzero upper-triangular k-pairs (42 of 72 tile-pairs,
at fp8 scale 1024). The diagonal term and s*||x|| are folded into a tiny
host-computed [128,4] tensor yb = 1024*(s*||x|| + sum(d x^2)); then
1/Y_extra = 32*recip(2*qv + yb) scales the PSUM eviction (out in fp16).

All DRAM operands are pre-tiled host-side so every DMA is a [128 part x
contiguous-bytes] block.
"""

import os
from contextlib import ExitStack

import numpy as np
import ml_dtypes

import concourse.bass as bass
import concourse.tile as tile
from concourse import bacc, mybir
from concourse import bass_utils

B = 4096
IN_NUMEL = 3072
OUT_NUMEL = 16384
OUT_DIMS = (64, 16, 16)
N_CORES = 8
BC = B // N_CORES            # 512 batch columns per core
P = 128
NB = BC // P                 # 4 batch blocks of 128
NMC = OUT_NUMEL // 512       # 32 m-chunks
KP = IN_NUMEL // 256         # 12 k-pairs (256 rows each)
NKC = IN_NUMEL // 512        # 6 k'-chunks for GEMM2

CM = int(os.environ.get("KERNEL_CM", "9"))    # corrected pairs, M-side
CX = int(os.environ.get("KERNEL_CX", "9"))    # corrected pairs, X-side
T_SUBB = 2 * (CM + CX)                         # xsB subtiles

SCALE_M = 32.0
SCALE_T = 1024.0

E4 = ml_dtypes.float8_e4m3
FP8 = mybir.dt.float8e4
F16 = mybir.dt.float16
F32 = mybir.dt.float32

# GEMM2 upper-triangular pair packing: chunk c uses pairs 0..2c+1
U_NP = [2 * c + 2 for c in range(NKC)]
U_OFF = [sum(U_NP[:c]) for c in range(NKC)]
U_TOT = sum(U_NP)                              # 42

_PROGRAM = None


def _build_program():
    nc = bacc.Bacc(
        "TRN2",
        target_bir_lowering=False,
        debug=False,
        enable_asserts=False,
        num_devices=N_CORES,
        enable_partition_id=False,
    )
    xsa_d = nc.dram_tensor("xsa", (P, 2 * KP, BC), FP8, kind="ExternalInput")
    xsb_d = nc.dram_tensor("xsb", (P, T_SUBB, BC), FP8, kind="ExternalInput")
    xh_d = nc.dram_tensor("xh", (P, NB, IN_NUMEL), FP8, kind="ExternalInput")
    yb_d = nc.dram_tensor("yb", (P, NB), F32, kind="ExternalInput")
    m1_d = nc.dram_tensor("m1", (NMC, P, KP, 2, 512), FP8, kind="ExternalInput")
    dm_d = nc.dram_tensor("dm", (NMC, P, CM, 2, 512), FP8, kind="ExternalInput")
    u8_d = nc.dram_tensor("u8", (U_TOT, P, 2, 512), FP8, kind="ExternalInput")
    out_d = nc.dram_tensor("out", (BC, OUT_NUMEL), F16, kind="ExternalOutput")

    Alu = mybir.AluOpType
    DR = mybir.MatmulPerfMode.DoubleRow

    m1_ap = m1_d.ap()
    dm_ap = dm_d.ap()
    u8_ap = u8_d.ap().rearrange("n p two i -> p n two i")
    out_ap = out_d.ap().rearrange("(t p) m -> p t m", p=P)

    with tile.TileContext(nc) as tc:
        with ExitStack() as ctx:
            small = ctx.enter_context(tc.tile_pool(name="small", bufs=1))
            xs_pool = ctx.enter_context(tc.tile_pool(name="xsp", bufs=1))
            xh_pool = ctx.enter_context(tc.tile_pool(name="xhp", bufs=1))
            u_pool = ctx.enter_context(tc.tile_pool(name="up", bufs=1))
            mm_pool = ctx.enter_context(tc.tile_pool(name="mmp", bufs=3))
            scr_pool = ctx.enter_context(tc.tile_pool(name="scr", bufs=2))
            out_pool = ctx.enter_context(tc.tile_pool(name="outp", bufs=6))
            psg2 = ctx.enter_context(tc.psum_pool(name="psg2", bufs=3))
            psg1 = ctx.enter_context(tc.psum_pool(name="psg1", bufs=5))

            qp = small.tile([P, NB * NKC], F32)
            qv = small.tile([P, NB], F32)
            ye = small.tile([P, NB], F32)
            rt = small.tile([P, NB], F32)
            rt32 = small.tile([P, NB], F32)
            yb_t = small.tile([P, NB], F32)

            xsa_t = xs_pool.tile([P, 2 * KP, BC], FP8)
            xsb_t = xs_pool.tile([P, T_SUBB, BC], FP8)
            xh_t = xh_pool.tile([P, NB, IN_NUMEL], FP8)
            u_tiles = [
                u_pool.tile([P, U_NP[c], 2, 512], FP8, tag=f"u{c}", name=f"u{c}")
                for c in range(NKC)
            ]

            # Early DMAs, in the order they should hit the wire: each lands
            # just before the in-order PE stream needs it. xsa/m1_0 stream
            # as interleaved thirds so the chunk-0 main matmuls start ~5us
            # in; GEMM2 u-tiles stream largest-first (the chunk loop below
            # runs c=5..0 to match).
            m1_cur = mm_pool.tile([P, KP, 2, 512], FP8, tag="m1")
            dm_cur = mm_pool.tile([P, CM, 2, 512], FP8, tag="dm")
            # first j-pair alone so the very first matmul issues ~2us earlier
            nc.sync.dma_start(xsa_t[:, 0:2], xsa_d.ap()[:, 0:2])
            nc.sync.dma_start(m1_cur[:, 0:1], m1_ap[0][:, 0:1])
            for lo, hi in ((1, 4), (4, 8), (8, 12)):
                nc.sync.dma_start(
                    xsa_t[:, 2 * lo:2 * hi], xsa_d.ap()[:, 2 * lo:2 * hi]
                )
                nc.sync.dma_start(m1_cur[:, lo:hi], m1_ap[0][:, lo:hi])
            def dma_u(c, lo, hi):
                nc.sync.dma_start(
                    u_tiles[c][:, lo:hi], u8_ap[:, U_OFF[c] + lo:U_OFF[c] + hi]
                )

            dma_u(5, 0, 6)
            dma_u(5, 6, 12)
            nc.sync.dma_start(xh_t[:, 0:2], xh_d.ap()[:, 0:2])
            dma_u(4, 0, 5)
            nc.sync.dma_start(xh_t[:, 2:4], xh_d.ap()[:, 2:4])
            dma_u(4, 5, 10)
            nc.sync.dma_start(xsb_t[:, 2 * CM:], xsb_d.ap()[:, 2 * CM:])  # dXq
            dma_u(3, 0, 8)
            dma_u(2, 0, 6)
            dma_u(1, 0, 4)
            dma_u(0, 0, 2)
            nc.sync.dma_start(xsb_t[:, 0:2 * CM], xsb_d.ap()[:, 0:2 * CM])  # X8
            nc.sync.dma_start(dm_cur[:], dm_ap[0])
            nc.sync.dma_start(yb_t[:], yb_d.ap())
            m1_nxt = mm_pool.tile([P, KP, 2, 512], FP8, tag="m1")
            dm_nxt = mm_pool.tile([P, CM, 2, 512], FP8, tag="dm")
            nc.sync.dma_start(m1_nxt[:, 0:6], m1_ap[1][:, 0:6])
            nc.sync.dma_start(m1_nxt[:, 6:12], m1_ap[1][:, 6:12])
            nc.sync.dma_start(dm_nxt[:], dm_ap[1])

            # GEMM2: qp[b,c] = sum((U^T X)[:,512c:512c+512] * x_nat)
            def gemm2_chunk(c):
                u_t = u_tiles[c]
                for b in range(NB):
                    ps = psg2.tile([P, 512], F32, tag="g2", name="psg2t")
                    npair = U_NP[c]
                    for j in range(npair):
                        nc.tensor.matmul(
                            ps[:],
                            xsa_t[:, 2 * j:2 * j + 2, b * P:(b + 1) * P],
                            u_t[:, j],
                            start=(j == 0),
                            stop=(j == npair - 1),
                            perf_mode=DR,
                        )
                    tmp = scr_pool.tile([P, 512], F32, tag="red", name="red")
                    nc.vector.tensor_tensor_reduce(
                        out=tmp[:], in0=ps[:],
                        in1=xh_t[:, b, c * 512:(c + 1) * 512],
                        scale=2.0, scalar=0.0,
                        op0=Alu.mult, op1=Alu.add,
                        accum_out=qp[:, b * NKC + c:b * NKC + c + 1],
                    )

            # G1 chunk 0, main-term matmuls (j-major to match the arriving
            # m1 thirds). The 4 PSUM groups stay open (no stop) until the
            # corrections are appended between/after the GEMM2 chunks.
            ps0 = [
                psg1.tile([P, 512], F32, tag="g1", name=f"ps0_{b}")
                for b in range(NB)
            ]
            for j in range(KP):
                for b in range(NB):
                    nc.tensor.matmul(
                        ps0[b][:],
                        xsa_t[:, 2 * j:2 * j + 2, b * P:(b + 1) * P],
                        m1_cur[:, j],
                        start=(j == 0), stop=False, perf_mode=DR,
                    )

            gemm2_chunk(5)
            gemm2_chunk(4)
            # chunk-0 X-side corrections: slot between the big and small
            # GEMM2 chunks to match the DMA arrival order (xsb lands after
            # u5/u4, before u3..u0).
            for b in range(NB):
                bsl = slice(b * P, (b + 1) * P)
                for j in range(CX):
                    nc.tensor.matmul(
                        ps0[b][:],
                        xsb_t[:, 2 * CM + 2 * j:2 * CM + 2 * j + 2, bsl],
                        m1_cur[:, j],
                        start=False, stop=False, perf_mode=DR,
                    )
            for c in (3, 2, 1, 0):
                gemm2_chunk(c)

            # 1/Y_extra (at combined scale): rt32 = 32 / (qv + yb); the 2x on
            # the strict-upper quadratic term is folded into the reducers.
            for b in range(NB):
                nc.vector.tensor_reduce(
                    qv[:, b:b + 1], qp[:, b * NKC:(b + 1) * NKC],
                    mybir.AxisListType.X, Alu.add,
                )
            nc.vector.scalar_tensor_tensor(
                out=ye[:], in0=qv[:], scalar=1.0, in1=yb_t[:],
                op0=Alu.mult, op1=Alu.add,
            )
            nc.vector.reciprocal(rt[:], ye[:])
            nc.vector.tensor_scalar_mul(rt32[:], rt[:], float(SCALE_M))

            # GEMM1: 32 m-chunks x 4 b-subtiles; 12 main + CX + CM fp8-DR
            # matmuls accumulate into one PSUM bank, evicted as out*rt32.
            def corr1_and_evict(ps, b, mu, dm_t):
                bsl = slice(b * P, (b + 1) * P)
                for j in range(CM):           # dM8 @ X8 (M-side residual)
                    nc.tensor.matmul(
                        ps[:], xsb_t[:, 2 * j:2 * j + 2, bsl], dm_t[:, j],
                        start=False, stop=(j == CM - 1), perf_mode=DR,
                    )
                ot = out_pool.tile([P, 512], F16, name="ot")
                if b % 2:    # split evictions across DVE and the idle ACT
                    nc.scalar.mul(ot[:], ps[:], rt32[:, b:b + 1])
                else:
                    nc.vector.tensor_scalar_mul(ot[:], ps[:], rt32[:, b:b + 1])
                nc.sync.dma_start(out_ap[:, b, mu * 512:(mu + 1) * 512], ot[:])

            def corr_and_evict(ps, b, mu, m1_t, dm_t):
                bsl = slice(b * P, (b + 1) * P)
                for j in range(CX):           # M1 @ dXq (X-side residual)
                    nc.tensor.matmul(
                        ps[:],
                        xsb_t[:, 2 * CM + 2 * j:2 * CM + 2 * j + 2, bsl],
                        m1_t[:, j],
                        start=False, stop=False, perf_mode=DR,
                    )
                corr1_and_evict(ps, b, mu, dm_t)

            for b in range(NB):               # finish chunk 0
                corr1_and_evict(ps0[b], b, 0, dm_cur)
            m1_cur, dm_cur = m1_nxt, dm_nxt

            for mu in range(1, NMC):
                if mu + 1 < NMC:
                    m1_nxt = mm_pool.tile([P, KP, 2, 512], FP8, tag="m1")
                    dm_nxt = mm_pool.tile([P, CM, 2, 512], FP8, tag="dm")
                    nc.sync.dma_start(m1_nxt[:], m1_ap[mu + 1])
                    nc.sync.dma_start(dm_nxt[:], dm_ap[mu + 1])
                for b in range(NB):
                    bsl = slice(b * P, (b + 1) * P)
                    ps = psg1.tile([P, 512], F32, tag="g1")
                    for j in range(KP):
                        nc.tensor.matmul(
                            ps[:], xsa_t[:, 2 * j:2 * j + 2, bsl], m1_cur[:, j],
                            start=(j == 0), stop=False, perf_mode=DR,
                        )
                    corr_and_evict(ps, b, mu, m1_cur, dm_cur)
                if mu + 1 < NMC:
                    m1_cur, dm_cur = m1_nxt, dm_nxt

    nc.compile()
    return nc


def get_program():
    global _PROGRAM
    if _PROGRAM is None:
        _PROGRAM = _build_program()
    return _PROGRAM


def _f8(a):
    return np.asarray(a, dtype=np.float32).astype(E4)


def make_in_maps(x, cached_matrix, cached_matrix_extra, cached_tensor_extra):
    xf = np.ascontiguousarray(np.asarray(x, dtype=np.float32).reshape(B, IN_NUMEL))
    s = float(np.asarray(cached_matrix_extra).reshape(-1)[0])

    # --- replicated model-cache tensors ---
    MT = np.ascontiguousarray(np.asarray(cached_matrix, dtype=np.float32).T)
    M32 = SCALE_M * MT
    M1 = _f8(M32)                                    # [3072, 16384] fp8
    dM8 = _f8(8.0 * (M32 - M1.astype(np.float32)))[: 256 * CM]
    # pre-tile: k = j*256 + tw*128 + p, m = mu*512 + i -> [mu, p, j, tw, i]
    m1_t = np.ascontiguousarray(
        M1.reshape(KP, 2, P, NMC, 512).transpose(3, 2, 0, 1, 4)
    )
    dm_t = np.ascontiguousarray(
        dM8.reshape(CM, 2, P, NMC, 512).transpose(3, 2, 0, 1, 4)
    )

    T0 = np.asarray(cached_tensor_extra, dtype=np.float32)
    S = 0.5 * (T0 + T0.T)
    d = np.diag(S).astype(np.float64).copy()
    U8 = _f8(SCALE_T * np.triu(S, 1))                # [3072, 3072] fp8
    u_parts = []
    for c in range(NKC):
        for j in range(U_NP[c]):
            blk = U8[256 * j:256 * (j + 1), 512 * c:512 * (c + 1)]
            u_parts.append(blk.reshape(2, P, 512).transpose(1, 0, 2))
    u8_t = np.ascontiguousarray(np.stack(u_parts, axis=0))  # [42, 128, 2, 512]

    # --- per-core batch-sharded tensors ---
    x64 = xf.astype(np.float64)
    yb_full = SCALE_T * (
        s * np.sqrt(np.sum(x64 * x64, axis=1))
        + np.sum(x64 * x64 * d[None, :], axis=1)
    ).astype(np.float64)                              # [B]

    in_maps = []
    for cidx in range(N_CORES):
        sl = slice(cidx * BC, (cidx + 1) * BC)
        Xc = np.ascontiguousarray(xf[sl].T)          # [3072, 512] f32
        X1 = _f8(Xc)
        X8 = _f8(X1.astype(np.float32) / 8.0)[: 256 * CM]
        dXq = _f8(Xc - X1.astype(np.float32))[: 256 * CX]
        xsa = np.ascontiguousarray(
            X1.reshape(2 * KP, P, BC).transpose(1, 0, 2)
        )
        xsb = np.ascontiguousarray(
            np.concatenate([X8, dXq], axis=0).reshape(T_SUBB, P, BC).transpose(1, 0, 2)
        )
        xh = np.ascontiguousarray(
            _f8(xf[sl]).reshape(NB, P, IN_NUMEL).transpose(1, 0, 2)
        )
        yb = np.ascontiguousarray(
            yb_full[sl].astype(np.float32).reshape(NB, P).T
        )
        in_maps.append({
            "xsa": xsa,
            "xsb": xsb,
            "xh": xh,
            "yb": yb,
            "m1": m1_t,
            "dm": dm_t,
            "u8": u8_t,
        })
    return in_maps


_AXON_EXEC = None
_SHARDED_INPUTS = {"xsa", "xsb", "xh", "yb"}


def _build_axon_exec():
    """Staged PJRT runner for the axon path.

    run_bass_kernel_spmd's axon redirect concatenates all per-core inputs into
    single giant host arrays for the replicated model caches, which hits a
    pathologically slow transfer path in the relay. Instead we stage shards/
    replicas with individually-sized device_puts and run the same bass_exec
    custom call through shard_map ourselves.
    """
    import jax
    from jax.sharding import Mesh, NamedSharding, PartitionSpec
    from jax.experimental.shard_map import shard_map
    from concourse import bass2jax

    nc = get_program()
    bass2jax.install_neuronx_cc_hook()

    in_names, out_names, out_avals = [], [], []
    for alloc in nc.m.functions[0].allocations:
        if not isinstance(alloc, mybir.MemoryLocationSet):
            continue
        name = alloc.memorylocations[0].name
        if alloc.kind == "ExternalInput":
            in_names.append(name)
        elif alloc.kind == "ExternalOutput":
            out_names.append(name)
            out_avals.append(
                jax.core.ShapedArray(
                    tuple(alloc.tensor_shape), mybir.dt.np(alloc.dtype)
                )
            )
    all_in_names = in_names + out_names

    def _body(*args):
        outs = bass2jax._bass_exec_p.bind(
            *args,
            out_avals=tuple(out_avals),
            in_names=tuple(all_in_names),
            out_names=tuple(out_names),
            lowering_input_output_aliases=(),
            sim_require_finite=True,
            sim_require_nnan=True,
            nc=nc,
        )
        return tuple(outs)

    devices = jax.devices()[:N_CORES]
    mesh = Mesh(np.asarray(devices), ("core",))
    core_spec = PartitionSpec("core")
    repl_spec = PartitionSpec()
    in_specs = tuple(
        core_spec if n in _SHARDED_INPUTS else repl_spec for n in in_names
    ) + (core_spec,) * len(out_names)
    sharded = jax.jit(
        shard_map(
            _body,
            mesh=mesh,
            in_specs=in_specs,
            out_specs=(core_spec,) * len(out_names),
            check_rep=False,
        ),
        keep_unused=True,
    )

    def stage(in_maps):
        import concurrent.futures as cf

        core_sh = NamedSharding(mesh, core_spec)
        repl_sh = NamedSharding(mesh, repl_spec)

        def stage_one(name):
            if name in _SHARDED_INPUTS:
                glob = np.concatenate([m[name] for m in in_maps], axis=0)
                return jax.device_put(glob, core_sh)
            return jax.device_put(in_maps[0][name], repl_sh)

        with cf.ThreadPoolExecutor(len(in_names)) as ex:
            staged = list(ex.map(stage_one, in_names))
        for st in staged:
            st.block_until_ready()
        zeros = [
            jax.jit(
                lambda a=a: jax.numpy.zeros(
                    (N_CORES * a.shape[0], *a.shape[1:]), a.dtype
                ),
                out_shardings=core_sh,
            )()
            for a in out_avals
        ]
        return staged + zeros

    def execute(staged):
        outs = sharded(*staged)
        jax.block_until_ready(outs)
        return outs

    def run(in_maps):
        return execute(stage(in_maps))

    return {"sharded": sharded, "stage": stage, "execute": execute, "run": run}


def get_axon_exec():
    global _AXON_EXEC
    if _AXON_EXEC is None:
        _AXON_EXEC = _build_axon_exec()
    return _AXON_EXEC


def kernel(x, cached_matrix, cached_matrix_extra, cached_tensor_extra):
    from concourse._compat import axon_active

    in_maps = make_in_maps(x, cached_matrix, cached_matrix_extra, cached_tensor_extra)
    if axon_active():
        outs = get_axon_exec()["run"](in_maps)
        out = np.asarray(outs[0])  # [B, OUT_NUMEL] f16
    else:
        nc = get_program()
        res = bass_utils.run_bass_kernel_spmd(nc, in_maps, core_ids=list(range(N_CORES)))
        out = np.concatenate([r["out"] for r in res.results], axis=0)
    return np.ascontiguousarray(out.astype(np.float32)).reshape(B, *OUT_DIMS)


# revision 34
# speedup vs baseline: 1.0350x; 1.0323x over previous
"""Trainium2 Bass kernel for nn_ConformalLayers (8-core data-parallel).

Math (reference):
    X = x.reshape(B, 3072).T                         # [3072, B]
    Y = M @ X                                        # [16384, B]
    Y_extra = s * ||X||_col + sum((T @ X) * X, 0)    # [1, B]
    out = (Y / Y_extra).T.reshape(B, 64, 16, 16)

Sharding: batch B=4096 split as 512 columns per core; model caches
replicated. Each core computes out^T rows [512, 16384] locally; host
concatenates.

All GEMMs run in fp8e4m3 with MatmulPerfMode.DoubleRow (2 k-tiles of 128
per instruction at 0.5 cycles/row = 4x the fp16 rate). fp8 quantization
of both operands alone gives ~3.2e-2 rel err (gate is 2e-2), so the main
GEMM adds two residual-correction streams over the first CM=9 / CX=8 of
12 k-pairs:

    sm*sx*M@X ~= M1@X1  +  dM8@X8  +  M1@dXq     (all fp8 DoubleRow)
      M1  = fp8(sm*M)          X1  = fp8(sx*X)
      dM8 = fp8(8*(sm*M - M1)) X8  = fp8(X1/8)    -> dM8@X8 ~ (smM-M1)@X1
      dXq = fp8(sx*X - X1)                        -> M1@dXq ~ M1@(sxX-X1)

    the 8/(1/8) pair cancels (the fp8 denormal range absorbs dXq's
    small magnitudes), so all three accumulate into one PSUM bank.
    Measured rel err 1.73e-2 on the real inputs. sm/sx/st are host-
    chosen powers of 2 (exponent shifts, bit-exact for the reference
    distributions; they only guard against rescaled harness inputs).

The quadratic form q = x^T T x uses only the symmetric part S=(T+T^T)/2:
q = 2 x^T U x + sum(diag(S) x^2) with U=triu(S,1), so the device GEMM2
streams only the nonzero upper-triangular k-pairs (42 of 72 tile-pairs,
at fp8 scale st). The diagonal term and s*||x|| are folded into a tiny
host-computed tensor yb = st*sx^2*(s*||x|| + sum(d x^2)); then
1/Y_extra-scaled eviction is psum * (st*sx/sm)*recip(qv + yb), out fp16.

All DRAM operands are pre-tiled host-side so every DMA is a [128 part x
contiguous-bytes] block; the startup DMA order is hand-tuned against the
TimelineSim so each transfer lands just before the in-order PE stream
needs it (GEMM2's u-chunks stream largest-first, chunk 0's main matmuls
interleave between GEMM2 chunks).
"""

import os
from contextlib import ExitStack

import numpy as np
import ml_dtypes

import concourse.bass as bass
import concourse.tile as tile
from concourse import bacc, mybir
from concourse import bass_utils

B = 4096
IN_NUMEL = 3072
OUT_NUMEL = 16384
OUT_DIMS = (64, 16, 16)
N_CORES = 8
BC = B // N_CORES            # 512 batch columns per core
P = 128
NB = BC // P                 # 4 batch blocks of 128
NMC = OUT_NUMEL // 512       # 32 m-chunks
KP = IN_NUMEL // 256         # 12 k-pairs (256 rows each)
NKC = IN_NUMEL // 512        # 6 k'-chunks for GEMM2

CM = int(os.environ.get("KERNEL_CM", "9"))    # corrected pairs, M-side
CX = int(os.environ.get("KERNEL_CX", "8"))    # corrected pairs, X-side
T_SUBB = 2 * (CM + CX)                         # xsB subtiles

E4 = ml_dtypes.float8_e4m3
FP8 = mybir.dt.float8e4
F16 = mybir.dt.float16
F32 = mybir.dt.float32

# GEMM2 upper-triangular pair packing: chunk c uses pairs 0..2c+1
U_NP = [2 * c + 2 for c in range(NKC)]
U_OFF = [sum(U_NP[:c]) for c in range(NKC)]
U_TOT = sum(U_NP)                              # 42

_PROGRAM = None


def _build_program():
    nc = bacc.Bacc(
        "TRN2",
        target_bir_lowering=False,
        debug=False,
        enable_asserts=False,
        num_devices=N_CORES,
        enable_partition_id=False,
    )
    xsa_d = nc.dram_tensor("xsa", (P, 2 * KP, BC), FP8, kind="ExternalInput")
    xsb_d = nc.dram_tensor("xsb", (P, T_SUBB, BC), FP8, kind="ExternalInput")
    xh_d = nc.dram_tensor("xh", (P, NB, IN_NUMEL), FP8, kind="ExternalInput")
    # yb carries NB per-batch-block Y_extra base terms plus, in its last
    # column, the host-chosen eviction scale SCALE_T*SX/SCALE_M.
    yb_d = nc.dram_tensor("yb", (P, NB + 1), F32, kind="ExternalInput")
    m1_d = nc.dram_tensor("m1", (NMC, P, KP, 2, 512), FP8, kind="ExternalInput")
    dm_d = nc.dram_tensor("dm", (NMC, P, CM, 2, 512), FP8, kind="ExternalInput")
    u8_d = nc.dram_tensor("u8", (U_TOT, P, 2, 512), FP8, kind="ExternalInput")
    out_d = nc.dram_tensor("out", (BC, OUT_NUMEL), F16, kind="ExternalOutput")

    Alu = mybir.AluOpType
    DR = mybir.MatmulPerfMode.DoubleRow

    m1_ap = m1_d.ap()
    dm_ap = dm_d.ap()
    u8_ap = u8_d.ap().rearrange("n p two i -> p n two i")
    out_ap = out_d.ap().rearrange("(t p) m -> p t m", p=P)

    with tile.TileContext(nc) as tc:
        with ExitStack() as ctx:
            small = ctx.enter_context(tc.tile_pool(name="small", bufs=1))
            xs_pool = ctx.enter_context(tc.tile_pool(name="xsp", bufs=1))
            xh_pool = ctx.enter_context(tc.tile_pool(name="xhp", bufs=1))
            u_pool = ctx.enter_context(tc.tile_pool(name="up", bufs=1))
            mm_pool = ctx.enter_context(tc.tile_pool(name="mmp", bufs=3))
            scr_pool = ctx.enter_context(tc.tile_pool(name="scr", bufs=2))
            out_pool = ctx.enter_context(tc.tile_pool(name="outp", bufs=6))
            psg2 = ctx.enter_context(tc.psum_pool(name="psg2", bufs=3))
            psg1 = ctx.enter_context(tc.psum_pool(name="psg1", bufs=5))

            qp = small.tile([P, NB * NKC], F32)
            qv = small.tile([P, NB], F32)
            ye = small.tile([P, NB], F32)
            rt = small.tile([P, NB], F32)
            rt32 = small.tile([P, NB], F32)
            yb_t = small.tile([P, NB + 1], F32)

            xsa_t = xs_pool.tile([P, 2 * KP, BC], FP8)
            xsb_t = xs_pool.tile([P, T_SUBB, BC], FP8)
            xh_t = xh_pool.tile([P, NB, IN_NUMEL], FP8)
            u_tiles = [
                u_pool.tile([P, U_NP[c], 2, 512], FP8, tag=f"u{c}", name=f"u{c}")
                for c in range(NKC)
            ]

            # Early DMAs, in the order they should hit the wire: each lands
            # just before the in-order PE stream needs it. xsa/m1_0 stream
            # as interleaved thirds so the chunk-0 main matmuls start ~5us
            # in; GEMM2 u-tiles stream largest-first (the chunk loop below
            # runs c=5..0 to match).
            m1_cur = mm_pool.tile([P, KP, 2, 512], FP8, tag="m1")
            dm_cur = mm_pool.tile([P, CM, 2, 512], FP8, tag="dm")
            # first j-pair alone so the very first matmul issues ~2us earlier
            nc.sync.dma_start(xsa_t[:, 0:2], xsa_d.ap()[:, 0:2])
            nc.sync.dma_start(m1_cur[:, 0:1], m1_ap[0][:, 0:1])
            for lo, hi in ((1, 4), (4, 8), (8, 12)):
                nc.sync.dma_start(
                    xsa_t[:, 2 * lo:2 * hi], xsa_d.ap()[:, 2 * lo:2 * hi]
                )
                nc.sync.dma_start(m1_cur[:, lo:hi], m1_ap[0][:, lo:hi])
            def dma_u(c, lo, hi):
                nc.sync.dma_start(
                    u_tiles[c][:, lo:hi], u8_ap[:, U_OFF[c] + lo:U_OFF[c] + hi]
                )

            dma_u(5, 0, 6)
            dma_u(5, 6, 12)
            nc.sync.dma_start(xh_t[:, 0:2], xh_d.ap()[:, 0:2])
            dma_u(4, 0, 5)
            nc.sync.dma_start(xh_t[:, 2:4], xh_d.ap()[:, 2:4])
            dma_u(4, 5, 10)
            nc.sync.dma_start(xsb_t[:, 2 * CM:], xsb_d.ap()[:, 2 * CM:])  # dXq
            dma_u(3, 0, 8)
            dma_u(2, 0, 6)
            dma_u(1, 0, 4)
            dma_u(0, 0, 2)
            nc.sync.dma_start(xsb_t[:, 0:2 * CM], xsb_d.ap()[:, 0:2 * CM])  # X8
            nc.sync.dma_start(dm_cur[:], dm_ap[0])
            nc.sync.dma_start(yb_t[:], yb_d.ap())
            m1_nxt = mm_pool.tile([P, KP, 2, 512], FP8, tag="m1")
            dm_nxt = mm_pool.tile([P, CM, 2, 512], FP8, tag="dm")
            nc.sync.dma_start(m1_nxt[:, 0:6], m1_ap[1][:, 0:6])
            nc.sync.dma_start(m1_nxt[:, 6:12], m1_ap[1][:, 6:12])
            nc.sync.dma_start(dm_nxt[:], dm_ap[1])

            # GEMM2: qp[b,c] = sum((U^T X)[:,512c:512c+512] * x_nat)
            def gemm2_chunk(c):
                u_t = u_tiles[c]
                for b in range(NB):
                    ps = psg2.tile([P, 512], F32, tag="g2", name="psg2t")
                    npair = U_NP[c]
                    for j in range(npair):
                        nc.tensor.matmul(
                            ps[:],
                            xsa_t[:, 2 * j:2 * j + 2, b * P:(b + 1) * P],
                            u_t[:, j],
                            start=(j == 0),
                            stop=(j == npair - 1),
                            perf_mode=DR,
                        )
                    tmp = scr_pool.tile([P, 512], F32, tag="red", name="red")
                    nc.vector.tensor_tensor_reduce(
                        out=tmp[:], in0=ps[:],
                        in1=xh_t[:, b, c * 512:(c + 1) * 512],
                        scale=2.0, scalar=0.0,
                        op0=Alu.mult, op1=Alu.add,
                        accum_out=qp[:, b * NKC + c:b * NKC + c + 1],
                    )

            # G1 chunk 0, main-term matmuls (j-major to match the arriving
            # m1 thirds). The 4 PSUM groups stay open (no stop) until the
            # corrections are appended between/after the GEMM2 chunks.
            ps0 = [
                psg1.tile([P, 512], F32, tag="g1", name=f"ps0_{b}")
                for b in range(NB)
            ]
            for j in range(KP):
                for b in range(NB):
                    nc.tensor.matmul(
                        ps0[b][:],
                        xsa_t[:, 2 * j:2 * j + 2, b * P:(b + 1) * P],
                        m1_cur[:, j],
                        start=(j == 0), stop=False, perf_mode=DR,
                    )

            gemm2_chunk(5)
            gemm2_chunk(4)
            # chunk-0 X-side corrections: slot between the big and small
            # GEMM2 chunks to match the DMA arrival order (xsb lands after
            # u5/u4, before u3..u0).
            for b in range(NB):
                bsl = slice(b * P, (b + 1) * P)
                for j in range(CX):
                    nc.tensor.matmul(
                        ps0[b][:],
                        xsb_t[:, 2 * CM + 2 * j:2 * CM + 2 * j + 2, bsl],
                        m1_cur[:, j],
                        start=False, stop=False, perf_mode=DR,
                    )
            for c in (3, 2, 1, 0):
                gemm2_chunk(c)

            # 1/Y_extra (at combined scale): rt32 = 32 / (qv + yb); the 2x on
            # the strict-upper quadratic term is folded into the reducers.
            for b in range(NB):
                nc.vector.tensor_reduce(
                    qv[:, b:b + 1], qp[:, b * NKC:(b + 1) * NKC],
                    mybir.AxisListType.X, Alu.add,
                )
            nc.vector.scalar_tensor_tensor(
                out=ye[:], in0=qv[:], scalar=1.0, in1=yb_t[:, 0:NB],
                op0=Alu.mult, op1=Alu.add,
            )
            nc.vector.reciprocal(rt[:], ye[:])
            nc.vector.tensor_scalar_mul(rt32[:], rt[:], yb_t[:, NB:NB + 1])

            # GEMM1: 32 m-chunks x 4 b-subtiles; 12 main + CX + CM fp8-DR
            # matmuls accumulate into one PSUM bank, evicted as out*rt32.
            def corr1_and_evict(ps, b, mu, dm_t):
                bsl = slice(b * P, (b + 1) * P)
                for j in range(CM):           # dM8 @ X8 (M-side residual)
                    nc.tensor.matmul(
                        ps[:], xsb_t[:, 2 * j:2 * j + 2, bsl], dm_t[:, j],
                        start=False, stop=(j == CM - 1), perf_mode=DR,
                    )
                ot = out_pool.tile([P, 512], F16, name="ot")
                if b % 2:    # split evictions across DVE and the idle ACT
                    nc.scalar.mul(ot[:], ps[:], rt32[:, b:b + 1])
                else:
                    nc.vector.tensor_scalar_mul(ot[:], ps[:], rt32[:, b:b + 1])
                nc.sync.dma_start(out_ap[:, b, mu * 512:(mu + 1) * 512], ot[:])

            def corr_and_evict(ps, b, mu, m1_t, dm_t):
                bsl = slice(b * P, (b + 1) * P)
                for j in range(CX):           # M1 @ dXq (X-side residual)
                    nc.tensor.matmul(
                        ps[:],
                        xsb_t[:, 2 * CM + 2 * j:2 * CM + 2 * j + 2, bsl],
                        m1_t[:, j],
                        start=False, stop=False, perf_mode=DR,
                    )
                corr1_and_evict(ps, b, mu, dm_t)

            for b in range(NB):               # finish chunk 0
                corr1_and_evict(ps0[b], b, 0, dm_cur)
            m1_cur, dm_cur = m1_nxt, dm_nxt

            for mu in range(1, NMC):
                if mu + 1 < NMC:
                    m1_nxt = mm_pool.tile([P, KP, 2, 512], FP8, tag="m1")
                    dm_nxt = mm_pool.tile([P, CM, 2, 512], FP8, tag="dm")
                    nc.sync.dma_start(m1_nxt[:], m1_ap[mu + 1])
                    nc.sync.dma_start(dm_nxt[:], dm_ap[mu + 1])
                for b in range(NB):
                    bsl = slice(b * P, (b + 1) * P)
                    ps = psg1.tile([P, 512], F32, tag="g1")
                    for j in range(KP):
                        nc.tensor.matmul(
                            ps[:], xsa_t[:, 2 * j:2 * j + 2, bsl], m1_cur[:, j],
                            start=(j == 0), stop=False, perf_mode=DR,
                        )
                    corr_and_evict(ps, b, mu, m1_cur, dm_cur)
                if mu + 1 < NMC:
                    m1_cur, dm_cur = m1_nxt, dm_nxt

    nc.compile()
    return nc


def get_program():
    global _PROGRAM
    if _PROGRAM is None:
        _PROGRAM = _build_program()
    return _PROGRAM


def _f8(a):
    return np.asarray(a, dtype=np.float32).astype(E4)


def _pow2_scale(maxval, target=224.0):
    """Largest power of 2 s.t. scale*maxval <= target (e4m3 max is 240).

    Power-of-2 scaling shifts fp8 exponents exactly, so for the reference
    distributions this is bit-equivalent to any fixed choice; it only
    guards against differently-scaled inputs."""
    import math
    maxval = float(maxval)
    if not np.isfinite(maxval) or maxval <= 0.0:
        return 1.0
    e = math.floor(math.log2(target / maxval))
    return float(2.0 ** max(-40, min(40, e)))


def make_in_maps(x, cached_matrix, cached_matrix_extra, cached_tensor_extra):
    xf = np.ascontiguousarray(np.asarray(x, dtype=np.float32).reshape(B, IN_NUMEL))
    s = float(np.asarray(cached_matrix_extra).reshape(-1)[0])

    # --- replicated model-cache tensors ---
    MT = np.ascontiguousarray(np.asarray(cached_matrix, dtype=np.float32).T)
    scale_m = _pow2_scale(np.abs(MT).max())
    M32 = scale_m * MT
    M1 = _f8(M32)                                    # [3072, 16384] fp8
    dM8 = _f8(8.0 * (M32 - M1.astype(np.float32)))[: 256 * CM]
    # pre-tile: k = j*256 + tw*128 + p, m = mu*512 + i -> [mu, p, j, tw, i]
    m1_t = np.ascontiguousarray(
        M1.reshape(KP, 2, P, NMC, 512).transpose(3, 2, 0, 1, 4)
    )
    dm_t = np.ascontiguousarray(
        dM8.reshape(CM, 2, P, NMC, 512).transpose(3, 2, 0, 1, 4)
    )

    T0 = np.asarray(cached_tensor_extra, dtype=np.float32)
    S = 0.5 * (T0 + T0.T)
    d = np.diag(S).astype(np.float64).copy()
    U = np.triu(S, 1)
    scale_t = _pow2_scale(np.abs(U).max())
    U8 = _f8(scale_t * U)                            # [3072, 3072] fp8
    u_parts = []
    for c in range(NKC):
        for j in range(U_NP[c]):
            blk = U8[256 * j:256 * (j + 1), 512 * c:512 * (c + 1)]
            u_parts.append(blk.reshape(2, P, 512).transpose(1, 0, 2))
    u8_t = np.ascontiguousarray(np.stack(u_parts, axis=0))  # [42, 128, 2, 512]

    # --- per-core batch-sharded tensors ---
    sx = _pow2_scale(np.abs(xf).max())
    smul = np.float32(scale_t * sx / scale_m)        # eviction scale
    x64 = xf.astype(np.float64)
    yb_full = (scale_t * sx * sx) * (
        s * np.sqrt(np.sum(x64 * x64, axis=1))
        + np.sum(x64 * x64 * d[None, :], axis=1)
    )                                                 # [B]

    in_maps = []
    for cidx in range(N_CORES):
        sl = slice(cidx * BC, (cidx + 1) * BC)
        Xc = np.ascontiguousarray(xf[sl].T) * np.float32(sx)  # [3072, 512]
        X1 = _f8(Xc)
        X8 = _f8(X1.astype(np.float32) / 8.0)[: 256 * CM]
        dXq = _f8(Xc - X1.astype(np.float32))[: 256 * CX]
        xsa = np.ascontiguousarray(
            X1.reshape(2 * KP, P, BC).transpose(1, 0, 2)
        )
        xsb = np.ascontiguousarray(
            np.concatenate([X8, dXq], axis=0).reshape(T_SUBB, P, BC).transpose(1, 0, 2)
        )
        xh = np.ascontiguousarray(
            _f8(sx * xf[sl]).reshape(NB, P, IN_NUMEL).transpose(1, 0, 2)
        )
        yb = np.concatenate(
            [
                yb_full[sl].astype(np.float32).reshape(NB, P).T,
                np.full((P, 1), smul, dtype=np.float32),
            ],
            axis=1,
        )
        in_maps.append({
            "xsa": xsa,
            "xsb": xsb,
            "xh": xh,
            "yb": np.ascontiguousarray(yb),
            "m1": m1_t,
            "dm": dm_t,
            "u8": u8_t,
        })
    return in_maps


_AXON_EXEC = None
_SHARDED_INPUTS = {"xsa", "xsb", "xh", "yb"}


def _build_axon_exec():
    """Staged PJRT runner for the axon path.

    run_bass_kernel_spmd's axon redirect concatenates all per-core inputs into
    single giant host arrays for the replicated model caches, which hits a
    pathologically slow transfer path in the relay. Instead we stage shards/
    replicas with individually-sized device_puts and run the same bass_exec
    custom call through shard_map ourselves.
    """
    import jax
    from jax.sharding import Mesh, NamedSharding, PartitionSpec
    from jax.experimental.shard_map import shard_map
    from concourse import bass2jax

    nc = get_program()
    bass2jax.install_neuronx_cc_hook()

    in_names, out_names, out_avals = [], [], []
    for alloc in nc.m.functions[0].allocations:
        if not isinstance(alloc, mybir.MemoryLocationSet):
            continue
        name = alloc.memorylocations[0].name
        if alloc.kind == "ExternalInput":
            in_names.append(name)
        elif alloc.kind == "ExternalOutput":
            out_names.append(name)
            out_avals.append(
                jax.core.ShapedArray(
                    tuple(alloc.tensor_shape), mybir.dt.np(alloc.dtype)
                )
            )
    all_in_names = in_names + out_names

    def _body(*args):
        outs = bass2jax._bass_exec_p.bind(
            *args,
            out_avals=tuple(out_avals),
            in_names=tuple(all_in_names),
            out_names=tuple(out_names),
            lowering_input_output_aliases=(),
            sim_require_finite=True,
            sim_require_nnan=True,
            nc=nc,
        )
        return tuple(outs)

    devices = jax.devices()[:N_CORES]
    mesh = Mesh(np.asarray(devices), ("core",))
    core_spec = PartitionSpec("core")
    repl_spec = PartitionSpec()
    in_specs = tuple(
        core_spec if n in _SHARDED_INPUTS else repl_spec for n in in_names
    ) + (core_spec,) * len(out_names)
    sharded = jax.jit(
        shard_map(
            _body,
            mesh=mesh,
            in_specs=in_specs,
            out_specs=(core_spec,) * len(out_names),
            check_rep=False,
        ),
        keep_unused=True,
    )

    def stage(in_maps):
        import concurrent.futures as cf

        core_sh = NamedSharding(mesh, core_spec)
        repl_sh = NamedSharding(mesh, repl_spec)

        def stage_one(name):
            if name in _SHARDED_INPUTS:
                glob = np.concatenate([m[name] for m in in_maps], axis=0)
                return jax.device_put(glob, core_sh)
            return jax.device_put(in_maps[0][name], repl_sh)

        with cf.ThreadPoolExecutor(len(in_names)) as ex:
            staged = list(ex.map(stage_one, in_names))
        for st in staged:
            st.block_until_ready()
        zeros = [
            jax.jit(
                lambda a=a: jax.numpy.zeros(
                    (N_CORES * a.shape[0], *a.shape[1:]), a.dtype
                ),
                out_shardings=core_sh,
            )()
            for a in out_avals
        ]
        return staged + zeros

    def execute(staged):
        outs = sharded(*staged)
        jax.block_until_ready(outs)
        return outs

    def run(in_maps):
        return execute(stage(in_maps))

    return {"sharded": sharded, "stage": stage, "execute": execute, "run": run}


def get_axon_exec():
    global _AXON_EXEC
    if _AXON_EXEC is None:
        _AXON_EXEC = _build_axon_exec()
    return _AXON_EXEC


def kernel(x, cached_matrix, cached_matrix_extra, cached_tensor_extra):
    from concourse._compat import axon_active

    in_maps = make_in_maps(x, cached_matrix, cached_matrix_extra, cached_tensor_extra)
    if axon_active():
        outs = get_axon_exec()["run"](in_maps)
        out = np.asarray(outs[0])  # [B, OUT_NUMEL] f16
    else:
        nc = get_program()
        res = bass_utils.run_bass_kernel_spmd(nc, in_maps, core_ids=list(range(N_CORES)))
        out = np.concatenate([r["out"] for r in res.results], axis=0)
    return np.ascontiguousarray(out.astype(np.float32)).reshape(B, *OUT_DIMS)
